# revision 38
# baseline (speedup 1.0000x reference)
"""Trainium2 Bass kernel: fused attention block (QKV proj + QK-norm + RoPE +
causal SDPA + out proj), tensor-parallel over 16 heads across 8 NeuronCores.

Layout strategy (all feature-major on device, zero on-device transposes):
  - host pre-packs every operand into the exact [partition, ...] layout its
    single merged DMA needs (each dma_start costs ~1.3us of issuing-engine
    sequencer time, so transfers are coalesced aggressively).
  - per core: q,k projected feature-major [d, tok]; v projected token-major
    and kept in SBUF (never leaves the core); scores computed transposed
    [k, tok_q] so softmax sums ride the PE; ctx is cast to bf16 and
    AllToAll'd per batch so comm overlaps the other batch's attention; each
    core then owns a 256-token slice per batch and runs the out-proj against
    the full w_out.T in bf16.
  - QK-norm gains (+ the 1/sqrt(d) score scale) are folded into host-made
    rope tables; RMS rstd comes from Square -> ones-matmul -> 1/sqrt(|x|)
    activation, broadcast with a ones-row matmul, applied after the
    (linear) rotation so no DVE op reads two PSUM operands.
  - RoPE: head dim host-permuted in 16-wide even/odd interleave so the
    rotation partner swap is one DVE stream_shuffle within 32-partition
    quadrants.
  - x/w_qkv/w_out and the whole softmax P / v / out-proj path run bf16
    (full PE rate at any free size); q/k stay f32r.
  - causal diagonal 512-blocks are narrowed per 128-key chunk.
  - attention runs a depth-2 software pipeline across head/batch boundaries;
    per-batch AllToAll + the next out-proj operand pulls are emitted from
    the pipeline drain events, ring-ordered so in-order HWDGE waits never
    block a later, earlier-firing transfer.
"""
import sys

sys.path.insert(0, "/opt/trn_rl_repo")
import numpy as np

import concourse.bacc as bacc
import concourse.mybir as mybir
from concourse.bass_utils import run_bass_kernel_spmd
from concourse.tile import TileContext

F32 = mybir.dt.float32
F32R = mybir.dt.float32r
BF16 = mybir.dt.bfloat16
AF = mybir.ActivationFunctionType

NCORES = 8
B, N, DM = 2, 2048, 2048
H, D = 16, 128
HLOC = H // NCORES          # 2 heads per core
T = B * N                   # 4096 flattened tokens
TCH = 8                     # token chunks of 512
KKN = DM // 128             # 16 dm chunks
HSL = N // NCORES           # 256 tokens per core per batch after a2a

SWAP16 = [(i + 16) % 32 for i in range(32)]  # rope partner swap mask

_CACHED = {}


def build():
    if "nc" in _CACHED:
        return _CACHED["nc"]
    nc = bacc.Bacc("TRN2", target_bir_lowering=False)
    _eps = nc.alloc_sbuf_tensor("const-eps", [128, 1], F32)
    nc.gpsimd.memset(_eps.ap(), 1e-6)
    nc.const_aps.aps[(F32, 1e-6)] = _eps.ap()
    nc.all_engine_barrier()

    xT = nc.dram_tensor("xT", [128, KKN, T], BF16, kind="ExternalInput")
    wqk = nc.dram_tensor("wqk", [128, KKN, 4 * D], BF16, kind="ExternalInput")
    wv = nc.dram_tensor("wv", [128, KKN, 2 * D], BF16, kind="ExternalInput")
    wo = nc.dram_tensor("wo", [128, 4, KKN, 512], BF16, kind="ExternalInput")
    csd = nc.dram_tensor("csd", [128, 4, N], F32, kind="ExternalInput")
    masks = nc.dram_tensor("masks", [128, 4 * 512], BF16, kind="ExternalInput")
    ones_col = nc.dram_tensor("ones_col", [128, 1], F32R, kind="ExternalInput")
    ones_colb = nc.dram_tensor("ones_colb", [128, 1], BF16, kind="ExternalInput")
    ones_row = nc.dram_tensor("ones_row", [1, 128], F32R, kind="ExternalInput")
    # out[b, p, tt, oc, c] = y[b, core*256 + tt*128 + p, oc*512 + c]
    out = nc.dram_tensor("out", [B, 128, 2, 4, 512], F32, kind="ExternalOutput")

    with TileContext(nc) as tc, nc.allow_low_precision(reason="bf16/f32r storage"):
        with (
            tc.tile_pool(name="acts", bufs=1) as acts,
            tc.tile_pool(name="dram", bufs=1, space="DRAM") as dpool,
        ):
            onc = acts.tile([128, 1], F32R, tag="onc")
            nc.sync.dma_start(onc[:], ones_col[:])
            oncb = acts.tile([128, 1], BF16, tag="oncb")
            nc.sync.dma_start(oncb[:], ones_colb[:])
            onr = acts.tile([1, 128], F32R, tag="onr")
            nc.sync.dma_start(onr[:], ones_row[:])

            # one exchange per (batch, head): smaller collectives start as
            # soon as that head's attention drains, keeping the (serialized)
            # collective cores busy from the earliest possible moment
            a2a_in = [[dpool.tile([1024, HSL], BF16, tag=f"a2a_in{b}_{h}",
                                  name=f"a2a_in{b}_{h}") for h in range(HLOC)]
                      for b in range(B)]
            a2a_out = [[dpool.tile([1024, HSL], BF16, tag=f"a2a_out{b}_{h}",
                                   name=f"a2a_out{b}_{h}") for h in range(HLOC)]
                       for b in range(B)]

            with tc.tile_pool(name="qkp", bufs=1) as qkp:
                qf = [qkp.tile([128, TCH, 512], F32R, tag=f"qf{h}", name=f"qf{h}")
                      for h in range(HLOC)]
                kf = [qkp.tile([128, TCH, 512], F32R, tag=f"kf{h}", name=f"kf{h}")
                      for h in range(HLOC)]
                # v lives in SBUF only: [tok-part, tok-chunk, head-feature]
                vbs = [qkp.tile([128, 16, 2 * D], BF16, tag=f"vb{b}", name=f"vb{b}")
                       for b in range(B)]
                masks_t = qkp.tile([128, 4 * 512], BF16, tag="masks")

                # ============ Phase A: QKV projection + QK-norm + RoPE ==========
                with (
                    tc.tile_pool(name="wts", bufs=1) as wts,
                    tc.tile_pool(name="xbp", bufs=2) as xbp,
                    tc.tile_pool(name="psA", bufs=2, space="PSUM") as psA,
                    tc.tile_pool(name="psS1", bufs=1, space="PSUM") as psS1,
                    tc.tile_pool(name="psB2", bufs=1, space="PSUM") as psB2,
                    tc.tile_pool(name="nrm", bufs=2) as nrm,
                    tc.tile_pool(name="ropes", bufs=2) as ropes,
                ):
                    wqk_t = wts.tile([128, KKN, 4 * D], BF16, tag="wqk")
                    for q4 in range(4):
                        nc.scalar.dma_start(wqk_t[:, 4 * q4:4 * (q4 + 1)],
                                            wqk[:, 4 * q4:4 * (q4 + 1)])
                    wv_t = wts.tile([128, KKN, 2 * D], BF16, tag="wv")
                    nc.scalar.dma_start(wv_t[:], wv[:])
                    nc.scalar.dma_start(masks_t[:], masks[:])

                    for tch in range(TCH):
                        pos = (tch % 4) * 512  # position offset within batch
                        cs = ropes.tile([128, 4, 512], F32, tag="cs")
                        nc.scalar.dma_start(cs[:], csd[:, :, pos:pos + 512])
                        xb = xbp.tile([128, KKN, 512], BF16, tag="xb")
                        if tch == 0:  # fine-grained so the PE can start early
                            for q4 in range(4):
                                nc.sync.dma_start(
                                    xb[:, 4 * q4:4 * (q4 + 1)],
                                    xT[:, 4 * q4:4 * (q4 + 1), 0:512])
                        else:
                            nc.sync.dma_start(xb[:], xT[:, :, tch * 512:(tch + 1) * 512])
                        # ---- q,k head-blocks: both pair-groups first ----
                        pqks = []
                        for ocp in range(2):
                            pqk = psA.tile([128, 2, 512], F32, tag="qk")
                            for kk in range(KKN):
                                for i in range(2):
                                    oc = 2 * ocp + i
                                    nc.tensor.matmul(
                                        pqk[:, i], wqk_t[:, kk, oc * 128:(oc + 1) * 128],
                                        xb[:, kk], start=(kk == 0), stop=(kk == KKN - 1))
                            pqks.append(pqk)

                        def qknorm(ocp):
                            # RMS-norm rstd + rope for the two ocs of group ocp
                            for i in range(2):
                                oc = 2 * ocp + i
                                ps = pqks[ocp][:, i]
                                sqr = nrm.tile([128, 512], F32R, tag="sq")
                                nc.scalar.activation(sqr[:], ps, AF.Square)
                                ssum = psS1.tile([1, 512], F32, tag="ssum")
                                nc.tensor.matmul(ssum[:], onc[:], sqr[:], start=True, stop=True)
                                rstd = nrm.tile([1, 512], F32R, tag="rstd")
                                nc.scalar.activation(rstd[:], ssum[:], AF.Abs_reciprocal_sqrt,
                                                     scale=1.0 / 128.0, bias=1e-6)
                                bcq = psB2.tile([128, 512], F32, tag="bcq")
                                nc.tensor.matmul(bcq[:], onr[:], rstd[:], start=True, stop=True)
                                # rope on the raw projection (linear per token
                                # column), rstd applied after; at most one
                                # PSUM operand per DVE op.
                                tt = nrm.tile([128, 512], F32, tag="tt")
                                nc.vector.stream_shuffle(tt[:], ps, SWAP16)
                                r1 = nrm.tile([128, 512], F32, tag="r1")
                                nc.vector.tensor_mul(r1[:], ps, cs[:, 2 * (oc // 2)])
                                nc.vector.tensor_mul(tt[:], tt[:], cs[:, 2 * (oc // 2) + 1])
                                nc.vector.tensor_add(r1[:], r1[:], tt[:])
                                dst = (qf[0], qf[1], kf[0], kf[1])[oc]
                                nc.vector.tensor_mul(dst[:, tch], r1[:], bcq[:])

                        qknorm(0)
                        qknorm(1)
                        # ---- v (token-major), copied straight into the
                        # persistent SBUF tile (no DRAM round trip); one
                        # accumulation group per PSUM region at a time ----
                        b_, c0 = tch // 4, (tch % 4) * 4
                        for tt in range(4):
                            pvt = psA.tile([128, 256], F32, tag="v")
                            for kk in range(KKN):
                                nc.tensor.matmul(
                                    pvt[:], xb[:, kk, tt * 128:(tt + 1) * 128],
                                    wv_t[:, kk], start=(kk == 0), stop=(kk == KKN - 1))
                            nc.scalar.copy(vbs[b_][:, c0 + tt], pvt[:])

                with (
                    tc.tile_pool(name="wop", bufs=1) as wop,
                    tc.tile_pool(name="cxp", bufs=1) as cxp,
                    tc.tile_pool(name="otp", bufs=2) as otp,
                ):
                    # w_out prefetch: resident through phase E; loads overlap
                    # the attention phase (one DMA per oc column block).
                    wot = wop.tile([128, 4, KKN, 512], BF16, tag="wo")
                    for oc in range(4):
                        nc.sync.dma_start(wot[:, oc], wo[:, oc])

                    # ============ Phase C: causal attention + split AllToAll ====
                    with (
                        tc.tile_pool(name="ctxpool", bufs=1) as ctxpool,
                        tc.tile_pool(name="Pp", bufs=4) as Pp,
                        tc.tile_pool(name="psS", bufs=2, space="PSUM") as psS,
                        tc.tile_pool(name="psM", bufs=2, space="PSUM") as psM,
                        tc.tile_pool(name="psN1", bufs=1, space="PSUM") as psN1,
                        tc.tile_pool(name="psB1", bufs=1, space="PSUM") as psB1,
                        tc.tile_pool(name="ctmp", bufs=2) as ctmp,
                    ):
                        ctx = [ctxpool.tile([128, TCH, 512], BF16, tag=f"ctx{h}",
                                            name=f"ctx{h}")
                               for h in range(HLOC)]
                        # narrowed free-ranges on the diagonal 512-block:
                        # scores (f32r moving needs >=256), exp/sums/ctx (bf16)
                        SC_LO = (0, 128, 256, 256)
                        EX_LO = (0, 128, 256, 384)

                        # cxt[(b, hh)] = [128, 8, 256] tile holding global
                        # heads {2c+hh} for batch b's own token slice
                        cxt = {}
                        cx_pending = []

                        def emit_cx(b_, hh_):
                            cxb = cxp.tile([128, NCORES, HSL], BF16,
                                           tag=f"cx{b_}_{hh_}", name=f"cx{b_}_{hh_}")
                            nc.sync.dma_start(
                                cxb[:],
                                a2a_out[b_][hh_].rearrange("(k p) c -> p k c", k=NCORES))
                            cxt[(b_, hh_)] = cxb

                        def emit_a2a(b_, hh_):
                            nc.scalar.dma_start(
                                a2a_in[b_][hh_].rearrange(
                                    "(q h p) c -> p q h c", q=4, h=2),
                                ctx[hh_][:, b_ * 4:(b_ + 1) * 4]
                                .rearrange("p q (h c) -> p q h c", h=2))
                            nc.gpsimd.collective_compute(
                                "AllToAll", mybir.AluOpType.bypass,
                                replica_groups=[list(range(NCORES))],
                                ins=[a2a_in[b_][hh_].opt()],
                                outs=[a2a_out[b_][hh_].opt()])
                            # previous unit's pull now; ours is queued so its
                            # collective-done wait can't delay later staging
                            while cx_pending:
                                emit_cx(*cx_pending.pop(0))
                            cx_pending.append((b_, hh_))

                        # chunk groups awaiting their sums/ctx matmuls
                        # (depth-2 software pipeline crossing head/batch
                        # boundaries)
                        pend = []

                        def flush_one():
                            halves, grp = pend.pop(0)
                            sums, ctxp, vb_, b_, hh_, tchq_ = grp
                            for P_ap, kk_, lo, last in halves:
                                nc.tensor.matmul(
                                    sums[:, lo:512], oncb[:], P_ap,
                                    start=(kk_ == 0), stop=last)
                                nc.tensor.matmul(
                                    ctxp[:, lo:512],
                                    vb_[:, kk_, hh_ * 128:(hh_ + 1) * 128],
                                    P_ap, start=(kk_ == 0), stop=last)
                                if last:  # qs complete: normalize
                                    rcp = ctmp.tile([1, 512], F32R, tag="rcp")
                                    nc.vector.reciprocal(rcp[:], sums[:])
                                    bc2 = psB1.tile([128, 512], F32, tag="bc2")
                                    nc.tensor.matmul(bc2[:], onr[:], rcp[:],
                                                     start=True, stop=True)
                                    bc2s = ctmp.tile([128, 512], F32, tag="bc2s")
                                    nc.vector.tensor_copy(bc2s[:], bc2[:])
                                    nc.vector.tensor_mul(
                                        ctx[hh_][:, tchq_], ctxp[:], bc2s[:])
                                    if tchq_ % 4 == 3:
                                        emit_a2a(b_, hh_)

                        for b in range(B):
                            vb = vbs[b]
                            for hh in range(HLOC):
                                for qs in range(4):
                                    tchq = b * 4 + qs
                                    sums = psN1.tile([1, 512], F32, tag="sums")
                                    ctxp = psM.tile([128, 512], F32, tag="ctxp")
                                    grp = (sums, ctxp, vb, b, hh, tchq)
                                    nsteps = 2 * qs + 4  # off-diag pairs + 4 diag
                                    for st in range(nsteps):
                                        if st < 2 * qs:  # off-diagonal pair
                                            sps = psS.tile([128, 2, 512], F32, tag="sps")
                                            for i in range(2):
                                                kk = 2 * st + i
                                                nc.tensor.matmul(
                                                    sps[:, i],
                                                    kf[hh][:, b * 4 + kk // 4,
                                                           (kk % 4) * 128:(kk % 4 + 1) * 128],
                                                    qf[hh][:, tchq], start=True, stop=True)
                                            P = Pp.tile([128, 2, 512], BF16, tag="P")
                                            nc.scalar.activation(P[:], sps[:], AF.Exp)
                                            halves = [(P[:, i], 2 * st + i, 0, False)
                                                      for i in range(2)]
                                        else:  # diagonal chunk, narrowed
                                            r = st - 2 * qs
                                            kk = 4 * qs + r
                                            slo, elo = SC_LO[r], EX_LO[r]
                                            sps = psS.tile([128, 2, 512], F32, tag="sps")
                                            nc.tensor.matmul(
                                                sps[:, 0, slo:512],
                                                kf[hh][:, b * 4 + kk // 4,
                                                       (kk % 4) * 128:(kk % 4 + 1) * 128],
                                                qf[hh][:, tchq, slo:512],
                                                start=True, stop=True)
                                            P = Pp.tile([128, 2, 512], BF16, tag="P")
                                            nc.scalar.activation(
                                                P[:, 0, elo:512], sps[:, 0, elo:512], AF.Exp)
                                            nc.vector.tensor_mul(
                                                P[:, 0, elo:512], P[:, 0, elo:512],
                                                masks_t[:, r * 512 + elo:(r + 1) * 512])
                                            halves = [(P[:, 0, elo:512], kk, elo, r == 3)]
                                        pend.append((halves, grp))
                                        if len(pend) > 2:
                                            flush_one()
                        while pend:
                            flush_one()
                        while cx_pending:
                            emit_cx(*cx_pending.pop(0))

                    # ================= Phase E: output projection ===============
                    with (
                        tc.tile_pool(name="psE", bufs=8, space="PSUM") as psE,
                    ):
                        # two passes per batch: every pso accumulates its 8
                        # even-head terms first (their exchange lands early),
                        # then the odd-head terms once the last collective
                        # arrives — the even pass fills that latency.
                        for b in range(B):
                            psos = []
                            for oc in range(4):
                                for tt in range(2):
                                    pso = psE.tile([128, 512], F32, tag="pso")
                                    for c_ in range(NCORES):
                                        nc.tensor.matmul(
                                            pso[:],
                                            cxt[(b, 0)][:, c_, tt * 128:(tt + 1) * 128],
                                            wot[:, oc, 2 * c_], start=(c_ == 0),
                                            stop=False)
                                    psos.append((pso, oc, tt))
                            for pso, oc, tt in psos:
                                for c_ in range(NCORES):
                                    nc.tensor.matmul(
                                        pso[:],
                                        cxt[(b, 1)][:, c_, tt * 128:(tt + 1) * 128],
                                        wot[:, oc, 2 * c_ + 1], start=False,
                                        stop=(c_ == NCORES - 1))
                                otb = otp.tile([128, 512], F32, tag="ot")
                                nc.scalar.copy(otb[:], pso[:])
                                nc.scalar.dma_start(out[b][:, tt, oc], otb[:])

    nc.compile()
    _CACHED["nc"] = nc
    return nc


def _host_inputs(x, w_qkv, w_out, qn_g, kn_g):
    import ml_dtypes

    x = np.asarray(x, dtype=np.float32)
    w_qkv = np.asarray(w_qkv, dtype=np.float32)
    w_out = np.asarray(w_out, dtype=np.float32)
    qn_g = np.asarray(qn_g, dtype=np.float32)
    kn_g = np.asarray(kn_g, dtype=np.float32)

    # head-dim permutation: 16-wide even/odd interleave so the rope partner
    # lives 16 partitions away within the same 32-partition quadrant
    perm = np.empty(D, dtype=np.int64)
    for q in range(4):
        perm[32 * q:32 * q + 16] = 2 * np.arange(16 * q, 16 * q + 16)      # evens
        perm[32 * q + 16:32 * q + 32] = 2 * np.arange(16 * q, 16 * q + 16) + 1

    bf16 = ml_dtypes.bfloat16
    xT = np.ascontiguousarray(
        x.reshape(T, DM).T.reshape(KKN, 128, T).transpose(1, 0, 2)).astype(bf16)
    wob = np.ascontiguousarray(
        w_out.T.reshape(KKN, 128, 4, 512).transpose(1, 2, 0, 3)).astype(bf16)

    # rope tables (position within a batch), permuted rows, gains folded.
    #   r[2i]   = t[2i] cos_i - t[2i+1] sin_i
    #   r[2i+1] = t[2i] sin_i + t[2i+1] cos_i
    inv = 1.0 / (10000.0 ** (np.arange(0, D, 2, dtype=np.float64) / D))  # [64]
    ang = np.arange(N, dtype=np.float64)[:, None] * inv[None, :]         # [N, 64]
    cosn = np.empty((D, N))
    sinn = np.empty((D, N))  # sign-folded sin for the partner term
    c = np.cos(ang).T        # [64, N] indexed by pair i
    s = np.sin(ang).T
    cosn[0::2] = c
    cosn[1::2] = c
    sinn[0::2] = -s          # even rows take -sin * partner(odd)
    sinn[1::2] = s
    qscale = 1.0 / np.sqrt(np.float64(D))
    partner_nat = np.arange(D) ^ 1

    def tables(g, scale):
        g = g.astype(np.float64)
        C = (cosn * g[:, None] * scale)[perm].astype(np.float32)
        S = ((sinn * g[partner_nat][:, None] * scale)[perm]).astype(np.float32)
        return C, S

    cqt, sqt = tables(qn_g, qscale)
    ckt, skt = tables(kn_g, 1.0)
    csd = np.ascontiguousarray(np.stack([cqt, sqt, ckt, skt], axis=1))  # [128,4,N]

    # causal diagonal masks, 4 variants r=0..3: allowed iff 128*r + p <= j
    p = np.arange(128)[:, None]
    j = np.arange(512)[None, :]
    masks = np.concatenate(
        [(128 * r + p <= j).astype(np.float32) for r in range(4)],
        axis=1).astype(bf16)

    shared = {
        "xT": xT, "wo": wob, "csd": csd, "masks": masks,
        "ones_col": np.ones((128, 1), np.float32),
        "ones_colb": np.ones((128, 1), bf16),
        "ones_row": np.ones((1, 128), np.float32),
    }
    in_maps = []
    for c_ in range(NCORES):
        hs = [HLOC * c_ + i for i in range(HLOC)]
        q_rows = np.concatenate([(0 * H + h) * D + perm for h in hs])
        k_rows = np.concatenate([(1 * H + h) * D + perm for h in hs])
        v_rows = np.concatenate([(2 * H + h) * D + np.arange(D) for h in hs])
        wqk_c = np.ascontiguousarray(
            w_qkv[np.concatenate([q_rows, k_rows]), :].T
            .reshape(KKN, 128, 512).transpose(1, 0, 2)).astype(bf16)
        wv_c = np.ascontiguousarray(
            w_qkv[v_rows, :].T.reshape(KKN, 128, 256).transpose(1, 0, 2)).astype(bf16)
        in_maps.append({**shared, "wqk": wqk_c, "wv": wv_c})
    return in_maps


def kernel(x, w_qkv, w_out, qn_g, kn_g):
    nc = build()
    in_maps = _host_inputs(x, w_qkv, w_out, qn_g, kn_g)
    res = run_bass_kernel_spmd(nc, in_maps, list(range(NCORES)))
    out = np.empty((B, N, DM), dtype=np.float32)
    for c in range(NCORES):
        o = res.results[c]["out"]  # [B, 128, 2, 4, 512]
        o = np.asarray(o).transpose(0, 2, 1, 3, 4).reshape(B, HSL, DM)
        out[:, c * HSL:(c + 1) * HSL, :] = o
    return out
